# revision 30
# baseline (speedup 1.0000x reference)
"""TRN2 Bass kernel for nn_Blur: upfirdn2d(pad=(2,1)) with a separable 4x4
binomial FIR, x shape (8, 256, 256, 256) f32, depthwise per (n, c) plane.

Strategy
--------
Batch-parallel across the 8 NeuronCores (core i gets x[i]).

The FIR is separable: out = T_H^T @ X @ T_W per (c) plane, where T_H/T_W are
256x256 banded Toeplitz matrices (band k1[0..3] on diagonals -1..+2, zero
boundary = the reference's zero padding).

Both passes run on the TensorEngine with the *data* as the stationary
operand (lhsT) and the Toeplitz as the moving operand (rhs):

  pass1:  Y^T = X^T @ T_H      (lhsT = X tile   [h_in=128, w=128],
                                rhs  = T_H blk  [h_in=128, h'=256])
  pass2:  Z   = Y  @ T_W       (lhsT = Y^T tile [w_in=128, h'=128],
                                rhs  = T_W blk  [w_in=128, w'=256])

so no transposes are needed: pass1 naturally yields Y^T, pass2 naturally
yields Z in output layout.

Precision: tolerance is 2e-2 relative, so the input is cast to plain bf16
on the host (quantization error ~2e-3 through the blur) and the Toeplitz
entries ([0.25, 0.75]) are exact in bf16. PSUM accumulates in fp32; the
Y^T intermediate is rounded to bf16 once more. Measured rel err ~1e-3.

DMA-efficiency tricks (descriptor size is what matters on TRN2):
 * inputs are pre-swizzled on the host into the exact SBUF tile layout
   [group][partition][c][hb][w] -> one 2 MiB DMA per group of CG=16
   channels with 16 KiB contiguous runs per partition.
 * the output DRAM tensor is partition-major [p][g][c][s][w] with
   h = s*128 + p, so stores are flat per-partition copies (8-16 KiB
   contiguous runs); the host un-swizzles afterwards.
 * input loads go out on the Sync (HWDGE) queue, stores on the GpSimd
   (SWDGE) queue, so stores never head-of-line-block loads.

Engine balance: the only non-PE compute is two PSUM->SBUF copies per
channel ([128, 512] each: Y^T round-to-bf16, Z round-to-bf16 staging);
they alternate between the Scalar and Vector engines so each engine
sees one copy per channel.
"""
import numpy as np
import ml_dtypes

import concourse.bacc as bacc
import concourse.mybir as mybir
from concourse.tile import TileContext
from concourse.bass_utils import run_bass_kernel_spmd

N, C, H, W = 8, 256, 256, 256
P = 128          # partition size
NCORES = 8
# band: T[i, i+d] = k1[d+1], d in {-1, 0, 1, 2}
BAND_LO, BAND_HI = -1, 2
# Both T_H and T_W are in natural order. Nonzero column spans per
# 128-row block: block0 (rows 0..127) -> cols [0, 130); block1 (rows
# 128..255) -> cols [127, 256); overlap [127, 130) needs both. The
# matmuls stream only nonzero columns, two per conv: PSUM's per-element
# has_written bit makes a single start=False matmul over [127, 256)
# accumulate on the overlap columns (bits set by the first matmul) and
# overwrite the rest (bits cleared by the first matmul's bank clear).
B0_HI = P + BAND_HI          # 130
OVL = (P + BAND_LO, P + BAND_HI)   # [127, 130)

CG = 16          # channels per DMA group

_CACHE = {}


def _factor_kernel(k2: np.ndarray):
    """Rank-1 factorization k2 = kh (x) kw (float64)."""
    k2 = np.asarray(k2, dtype=np.float64)
    u, s, vt = np.linalg.svd(k2)
    kh = u[:, 0] * np.sqrt(s[0])
    kw = vt[0] * np.sqrt(s[0])
    if kh.sum() < 0:
        kh, kw = -kh, -kw
    return kh, kw


def _toeplitz(n: int, k1: np.ndarray) -> np.ndarray:
    """T[i, j] = k1[j - i + 1] for 0 <= j-i+1 < 4, zero elsewhere."""
    t = np.zeros((n, n), dtype=np.float64)
    for d in range(BAND_LO, BAND_HI + 1):
        i = np.arange(max(0, -d), min(n, n - d))
        t[i, i + d] = k1[d + 1]
    return t


def _build(n_ch: int, cg: int = CG, reps: int = 1):
    """Build + compile the per-core Bass program (SPMD, one core's slice)."""
    nc = bacc.Bacc("TRN2", target_bir_lowering=False)

    bf16 = mybir.dt.bfloat16
    f32 = mybir.dt.float32

    assert n_ch % cg == 0
    ng = n_ch // cg
    # [group][partition][c][hb][w] pre-swizzled input, bf16
    xin = nc.declare_dram_parameter("xin", [ng, P, cg * 2 * W], bf16,
                                    isOutput=False)
    th = nc.declare_dram_parameter("th", [2, P, H], bf16, isOutput=False)
    tw = nc.declare_dram_parameter("tw", [2, P, W], bf16, isOutput=False)
    # partition-major output: [p][g][c][s][w] with h = 2p + s, so each
    # store is a flat per-partition copy with contiguous DRAM runs (the
    # host un-swizzles and upcasts afterwards). bf16 on the wire halves
    # store traffic; the f32 contract is restored host-side.
    out = nc.declare_dram_parameter("out", [P, ng, cg * 2 * W], bf16,
                                    isOutput=True)

    with TileContext(nc) as tc:
        with (tc.tile_pool(name="const", bufs=1) as cpool,
              tc.tile_pool(name="xin_p", bufs=4) as xpool,
              tc.tile_pool(name="mid", bufs=8) as mpool,
              tc.tile_pool(name="zout", bufs=4) as zpool,
              tc.tile_pool(name="psy", bufs=4, space="PSUM") as pypool,
              tc.tile_pool(name="psz", bufs=4, space="PSUM") as pzpool):

            tth = [cpool.tile([P, H], bf16, name=f"tth{b}", tag=f"tth{b}")
                   for b in range(2)]
            ttw = [cpool.tile([P, W], bf16, name=f"ttw{b}", tag=f"ttw{b}")
                   for b in range(2)]
            for b in range(2):
                nc.sync.dma_start(out=tth[b][:, :], in_=th[b])
                nc.sync.dma_start(out=ttw[b][:, :], in_=tw[b])

            first_g = True
            for g in [gg for _ in range(reps) for gg in range(ng)]:
                # one contiguous 2 MiB load: [128, 16 KiB]. The very first
                # group is loaded in 4 chunks so channel-0 compute starts
                # ~7 us earlier (region-tracked deps).
                tx = xpool.tile([P, cg * 2 * W], bf16, name="tx", tag="tx")
                if first_g or g == ng - 1:
                    # chunked loads: the first group so channel-0 compute
                    # starts early, the last group so tail compute trails
                    # each sub-load instead of the whole 2 MiB
                    first_g = False
                    q = cg * 2 * W // 4
                    for ch in range(4):
                        nc.sync.dma_start(out=tx[:, ch * q:(ch + 1) * q],
                                          in_=xin[g, :, ch * q:(ch + 1) * q])
                else:
                    nc.sync.dma_start(out=tx[:, :], in_=xin[g])

                tz = zpool.tile([P, cg * 2 * W], bf16, name="tz", tag="tz")

                for ci in range(cg):
                    # ---- pass1: Y^T[wb] = sum_hb X[hb,:,wb]^T @ TH[hb]
                    # one PSUM tile holds both wb halves: [128, 2*H] fp32
                    py = pypool.tile([P, 2 * H], f32, name="py", tag="py")
                    ty = mpool.tile([P, 2 * H], bf16, name="ty", tag="ty")
                    for wb in range(2):
                        base = wb * H
                        off0 = ci * 2 * W + 0 * W + wb * P
                        off1 = ci * 2 * W + 1 * W + wb * P
                        nc.tensor.matmul(
                            py[:, base:base + B0_HI], tx[:, off0:off0 + P],
                            tth[0][:, :B0_HI], start=True, stop=False)
                        nc.tensor.matmul(
                            py[:, base + OVL[0]:base + H],
                            tx[:, off1:off1 + P],
                            tth[1][:, OVL[0]:], start=False, stop=True)
                    # single [128, 512] PSUM->SBUF round-to-bf16 copy,
                    # alternated between Scalar and Vector (GpSimd has no
                    # PSUM port)
                    eng = ci % 2
                    if eng == 0:
                        nc.scalar.copy(ty[:, :], py[:, :])
                    else:
                        nc.vector.tensor_copy(ty[:, :], py[:, :])

                    # ---- pass2: Z[s] = sum_wb Y^T[wb,:,s]^T @ TW[wb]
                    # s-block split: partition p of s-group = output row
                    # h = s*128 + p.
                    pz = pzpool.tile([P, 2 * W], f32, name="pz", tag="pz")
                    for s in range(2):
                        zb = s * W
                        sl0 = slice(s * P, s * P + P)
                        sl1 = slice(H + s * P, H + s * P + P)
                        nc.tensor.matmul(
                            pz[:, zb:zb + B0_HI], ty[:, sl0],
                            ttw[0][:, :B0_HI], start=True, stop=False)
                        nc.tensor.matmul(
                            pz[:, zb + OVL[0]:zb + W], ty[:, sl1],
                            ttw[1][:, OVL[0]:], start=False, stop=True)
                    zsl = slice(ci * 2 * W, (ci + 1) * 2 * W)
                    if eng == 0:
                        nc.vector.tensor_copy(tz[:, zsl], pz[:, :])
                    else:
                        nc.scalar.copy(tz[:, zsl], pz[:, :])

                    # half-group store as soon as the first cg/2 channels
                    # are staged; SWDGE (GpSimd) queue so stores never
                    # head-of-line-block the Sync queue's input loads.
                    # The last group stores per quarter so the final store
                    # trails the tail compute as tightly as possible.
                    qrt = cg // 4
                    if g == ng - 1:
                        if (ci + 1) % qrt == 0:
                            kq = ci // qrt
                            csl = slice(kq * qrt * 2 * W,
                                        (kq + 1) * qrt * 2 * W)
                            nc.gpsimd.dma_start(out=out[:, g, csl],
                                                in_=tz[:, csl])
                    elif ci == cg // 2 - 1 or ci == cg - 1:
                        hlf = 0 if ci == cg // 2 - 1 else 1
                        csl = slice(hlf * cg * W, (hlf + 1) * cg * W)
                        nc.gpsimd.dma_start(out=out[:, g, csl],
                                            in_=tz[:, csl])
    nc.compile()
    return nc


def _get_nc(n_ch: int):
    key = (n_ch, CG)
    if key not in _CACHE:
        _CACHE[key] = _build(n_ch)
    return _CACHE[key]


def _prep_inputs(x: np.ndarray, k2: np.ndarray, n_ch: int):
    cg = CG
    ng = n_ch // cg
    kh, kw = _factor_kernel(k2)
    th64 = _toeplitz(H, kh)
    tw64 = _toeplitz(W, kw)
    th = th64.astype(ml_dtypes.bfloat16).reshape(2, P, H)
    tw = tw64.astype(ml_dtypes.bfloat16).reshape(2, P, W)
    th = np.ascontiguousarray(th)
    tw = np.ascontiguousarray(tw)

    xhi = np.asarray(x, dtype=np.float32).astype(ml_dtypes.bfloat16)
    # [n, c, h, w] -> [n, g, c', hb, p, w] -> [n, g, p, (c', hb, w)]
    xhi = xhi.reshape(N, ng, cg, 2, P, W)
    xin = xhi.transpose(0, 1, 4, 2, 3, 5)         # [n, g, p, c', hb, w]
    xin = np.ascontiguousarray(xin).reshape(N, ng, P, cg * 2 * W)

    in_maps = []
    for i in range(NCORES):
        in_maps.append({"xin": xin[i], "th": th, "tw": tw})
    return in_maps


def _run(x: np.ndarray, k2: np.ndarray, trace: bool = False):
    n_ch = C
    nc = _get_nc(n_ch)
    in_maps = _prep_inputs(x, k2, n_ch)
    r = run_bass_kernel_spmd(nc, in_maps, core_ids=list(range(NCORES)),
                             trace=trace)
    # out [P, ng, cg, 2, W] bf16: h = s*128 + p -> unswizzle + upcast to
    # [n_ch, H, W] f32
    ng = n_ch // CG
    outs = []
    for i in range(NCORES):
        o = r.results[i]["out"].reshape(P, ng, CG, 2, W)
        o = o.transpose(1, 2, 3, 0, 4).astype(np.float32)   # [g, c, s, p, w]
        outs.append(o.reshape(n_ch, H, W))
    return np.stack(outs, axis=0), r


def kernel(x: np.ndarray, kernel: np.ndarray) -> np.ndarray:
    out, _ = _run(x, kernel, trace=False)
    return out


# revision 31
# speedup vs baseline: 1.0349x; 1.0349x over previous
"""TRN2 Bass kernel for nn_Blur: upfirdn2d(pad=(2,1)) with a separable 4x4
binomial FIR, x shape (8, 256, 256, 256) f32, depthwise per (n, c) plane.

Strategy
--------
Batch-parallel across the 8 NeuronCores (core i gets x[i]).

The FIR is separable: out = T_H^T @ X @ T_W per (c) plane, where T_H/T_W are
256x256 banded Toeplitz matrices (band k1[0..3] on diagonals -1..+2, zero
boundary = the reference's zero padding).

Both passes run on the TensorEngine with the *data* as the stationary
operand (lhsT) and the Toeplitz as the moving operand (rhs):

  pass1:  Y^T = X^T @ T_H      (lhsT = X tile   [h_in=128, w=128],
                                rhs  = T_H blk  [h_in=128, h'=256])
  pass2:  Z   = Y  @ T_W       (lhsT = Y^T tile [w_in=128, h'=128],
                                rhs  = T_W blk  [w_in=128, w'=256])

so no transposes are needed: pass1 naturally yields Y^T, pass2 naturally
yields Z in output layout.

Precision: tolerance is 2e-2 relative, so the input is cast to plain bf16
on the host (quantization error ~2e-3 through the blur) and the Toeplitz
entries ([0.25, 0.75]) are exact in bf16. PSUM accumulates in fp32; the
Y^T intermediate is rounded to bf16 once more. Measured rel err ~1e-3.

DMA-efficiency tricks (descriptor size is what matters on TRN2):
 * inputs are pre-swizzled on the host into the exact SBUF tile layout
   [group][partition][c][hb][w] -> one 2 MiB DMA per group of CG=16
   channels with 16 KiB contiguous runs per partition.
 * the output DRAM tensor is partition-major [p][g][c][s][w] with
   h = s*128 + p, so stores are flat per-partition copies (8-16 KiB
   contiguous runs); the host un-swizzles afterwards.
 * input loads go out on the Sync (HWDGE) queue, stores on the GpSimd
   (SWDGE) queue, so stores never head-of-line-block loads.

Engine balance: the only non-PE compute is two PSUM->SBUF copies per
channel ([128, 512] each: Y^T round-to-bf16, Z round-to-bf16 staging);
they alternate between the Scalar and Vector engines so each engine
sees one copy per channel.
"""
import numpy as np
import ml_dtypes

import concourse.bacc as bacc
import concourse.mybir as mybir
from concourse.tile import TileContext
from concourse.bass_utils import run_bass_kernel_spmd

N, C, H, W = 8, 256, 256, 256
P = 128          # partition size
NCORES = 8
# band: T[i, i+d] = k1[d+1], d in {-1, 0, 1, 2}
BAND_LO, BAND_HI = -1, 2
# Both T_H and T_W are in natural order. Nonzero column spans per
# 128-row block: block0 (rows 0..127) -> cols [0, 130); block1 (rows
# 128..255) -> cols [127, 256); overlap [127, 130) needs both. The
# matmuls stream only nonzero columns, two per conv: PSUM's per-element
# has_written bit makes a single start=False matmul over [127, 256)
# accumulate on the overlap columns (bits set by the first matmul) and
# overwrite the rest (bits cleared by the first matmul's bank clear).
B0_HI = P + BAND_HI          # 130
OVL = (P + BAND_LO, P + BAND_HI)   # [127, 130)

CG = 16          # channels per DMA group

_CACHE = {}


def _factor_kernel(k2: np.ndarray):
    """Rank-1 factorization k2 = kh (x) kw (float64)."""
    k2 = np.asarray(k2, dtype=np.float64)
    u, s, vt = np.linalg.svd(k2)
    kh = u[:, 0] * np.sqrt(s[0])
    kw = vt[0] * np.sqrt(s[0])
    if kh.sum() < 0:
        kh, kw = -kh, -kw
    return kh, kw


def _toeplitz(n: int, k1: np.ndarray) -> np.ndarray:
    """T[i, j] = k1[j - i + 1] for 0 <= j-i+1 < 4, zero elsewhere."""
    t = np.zeros((n, n), dtype=np.float64)
    for d in range(BAND_LO, BAND_HI + 1):
        i = np.arange(max(0, -d), min(n, n - d))
        t[i, i + d] = k1[d + 1]
    return t


def _build(n_ch: int, cg: int = CG, reps: int = 1):
    """Build + compile the per-core Bass program (SPMD, one core's slice)."""
    nc = bacc.Bacc("TRN2", target_bir_lowering=False)

    bf16 = mybir.dt.bfloat16
    f32 = mybir.dt.float32

    assert n_ch % cg == 0
    ng = n_ch // cg
    # [group][partition][c][hb][w] pre-swizzled input, bf16
    xin = nc.declare_dram_parameter("xin", [ng, P, cg * 2 * W], bf16,
                                    isOutput=False)
    th = nc.declare_dram_parameter("th", [2, P, H], bf16, isOutput=False)
    tw = nc.declare_dram_parameter("tw", [2, P, W], bf16, isOutput=False)
    # partition-major output: [p][g][c][s][w] with h = 2p + s, so each
    # store is a flat per-partition copy with contiguous DRAM runs (the
    # host un-swizzles and upcasts afterwards). bf16 on the wire halves
    # store traffic; the f32 contract is restored host-side.
    out = nc.declare_dram_parameter("out", [P, ng, cg * 2 * W], bf16,
                                    isOutput=True)

    with TileContext(nc) as tc:
        with (tc.tile_pool(name="const", bufs=1) as cpool,
              tc.tile_pool(name="xin_p", bufs=4) as xpool,
              tc.tile_pool(name="mid", bufs=8) as mpool,
              tc.tile_pool(name="zout", bufs=4) as zpool,
              tc.tile_pool(name="psy", bufs=4, space="PSUM") as pypool,
              tc.tile_pool(name="psz", bufs=4, space="PSUM") as pzpool):

            tth = [cpool.tile([P, H], bf16, name=f"tth{b}", tag=f"tth{b}")
                   for b in range(2)]
            ttw = [cpool.tile([P, W], bf16, name=f"ttw{b}", tag=f"ttw{b}")
                   for b in range(2)]
            for b in range(2):
                nc.sync.dma_start(out=tth[b][:, :], in_=th[b])
                nc.sync.dma_start(out=ttw[b][:, :], in_=tw[b])

            first_g = True
            for g in [gg for _ in range(reps) for gg in range(ng)]:
                # one contiguous 2 MiB load: [128, 16 KiB]. The very first
                # group is loaded in 4 chunks so channel-0 compute starts
                # ~7 us earlier (region-tracked deps).
                tx = xpool.tile([P, cg * 2 * W], bf16, name="tx", tag="tx")
                if first_g:
                    # chunked first load so channel-0 compute starts early
                    first_g = False
                    q = cg * 2 * W // 4
                    for ch in range(4):
                        nc.sync.dma_start(out=tx[:, ch * q:(ch + 1) * q],
                                          in_=xin[g, :, ch * q:(ch + 1) * q])
                else:
                    nc.sync.dma_start(out=tx[:, :], in_=xin[g])

                tz = zpool.tile([P, cg * 2 * W], bf16, name="tz", tag="tz")

                for ci in range(cg):
                    # ---- pass1: Y^T[wb] = sum_hb X[hb,:,wb]^T @ TH[hb]
                    # one PSUM tile holds both wb halves: [128, 2*H] fp32
                    py = pypool.tile([P, 2 * H], f32, name="py", tag="py")
                    ty = mpool.tile([P, 2 * H], bf16, name="ty", tag="ty")
                    for wb in range(2):
                        base = wb * H
                        off0 = ci * 2 * W + 0 * W + wb * P
                        off1 = ci * 2 * W + 1 * W + wb * P
                        nc.tensor.matmul(
                            py[:, base:base + B0_HI], tx[:, off0:off0 + P],
                            tth[0][:, :B0_HI], start=True, stop=False)
                        nc.tensor.matmul(
                            py[:, base + OVL[0]:base + H],
                            tx[:, off1:off1 + P],
                            tth[1][:, OVL[0]:], start=False, stop=True)
                    # single [128, 512] PSUM->SBUF round-to-bf16 copy,
                    # alternated between Scalar and Vector (GpSimd has no
                    # PSUM port)
                    eng = ci % 2
                    if eng == 0:
                        nc.scalar.copy(ty[:, :], py[:, :])
                    else:
                        nc.vector.tensor_copy(ty[:, :], py[:, :])

                    # ---- pass2: Z[s] = sum_wb Y^T[wb,:,s]^T @ TW[wb]
                    # s-block split: partition p of s-group = output row
                    # h = s*128 + p.
                    pz = pzpool.tile([P, 2 * W], f32, name="pz", tag="pz")
                    for s in range(2):
                        zb = s * W
                        sl0 = slice(s * P, s * P + P)
                        sl1 = slice(H + s * P, H + s * P + P)
                        nc.tensor.matmul(
                            pz[:, zb:zb + B0_HI], ty[:, sl0],
                            ttw[0][:, :B0_HI], start=True, stop=False)
                        nc.tensor.matmul(
                            pz[:, zb + OVL[0]:zb + W], ty[:, sl1],
                            ttw[1][:, OVL[0]:], start=False, stop=True)
                    zsl = slice(ci * 2 * W, (ci + 1) * 2 * W)
                    if eng == 0:
                        nc.vector.tensor_copy(tz[:, zsl], pz[:, :])
                    else:
                        nc.scalar.copy(tz[:, zsl], pz[:, :])

                    # half-group store as soon as the first cg/2 channels
                    # are staged; SWDGE (GpSimd) queue so stores never
                    # head-of-line-block the Sync queue's input loads.
                    # The last group stores per quarter so the final store
                    # trails the tail compute as tightly as possible.
                    qrt = cg // 4
                    if g == ng - 1:
                        if (ci + 1) % qrt == 0:
                            kq = ci // qrt
                            csl = slice(kq * qrt * 2 * W,
                                        (kq + 1) * qrt * 2 * W)
                            nc.gpsimd.dma_start(out=out[:, g, csl],
                                                in_=tz[:, csl])
                    elif ci == cg // 2 - 1 or ci == cg - 1:
                        hlf = 0 if ci == cg // 2 - 1 else 1
                        csl = slice(hlf * cg * W, (hlf + 1) * cg * W)
                        nc.gpsimd.dma_start(out=out[:, g, csl],
                                            in_=tz[:, csl])
    nc.compile()
    return nc


def _get_nc(n_ch: int):
    key = (n_ch, CG)
    if key not in _CACHE:
        _CACHE[key] = _build(n_ch)
    return _CACHE[key]


def _prep_inputs(x: np.ndarray, k2: np.ndarray, n_ch: int):
    cg = CG
    ng = n_ch // cg
    kh, kw = _factor_kernel(k2)
    th64 = _toeplitz(H, kh)
    tw64 = _toeplitz(W, kw)
    th = th64.astype(ml_dtypes.bfloat16).reshape(2, P, H)
    tw = tw64.astype(ml_dtypes.bfloat16).reshape(2, P, W)
    th = np.ascontiguousarray(th)
    tw = np.ascontiguousarray(tw)

    xhi = np.asarray(x, dtype=np.float32).astype(ml_dtypes.bfloat16)
    # [n, c, h, w] -> [n, g, c', hb, p, w] -> [n, g, p, (c', hb, w)]
    xhi = xhi.reshape(N, ng, cg, 2, P, W)
    xin = xhi.transpose(0, 1, 4, 2, 3, 5)         # [n, g, p, c', hb, w]
    xin = np.ascontiguousarray(xin).reshape(N, ng, P, cg * 2 * W)

    in_maps = []
    for i in range(NCORES):
        in_maps.append({"xin": xin[i], "th": th, "tw": tw})
    return in_maps


def _run(x: np.ndarray, k2: np.ndarray, trace: bool = False):
    n_ch = C
    nc = _get_nc(n_ch)
    in_maps = _prep_inputs(x, k2, n_ch)
    r = run_bass_kernel_spmd(nc, in_maps, core_ids=list(range(NCORES)),
                             trace=trace)
    # out [P, ng, cg, 2, W] bf16: h = s*128 + p -> unswizzle + upcast to
    # [n_ch, H, W] f32
    ng = n_ch // CG
    outs = []
    for i in range(NCORES):
        o = r.results[i]["out"].reshape(P, ng, CG, 2, W)
        o = o.transpose(1, 2, 3, 0, 4).astype(np.float32)   # [g, c, s, p, w]
        outs.append(o.reshape(n_ch, H, W))
    return np.stack(outs, axis=0), r


def kernel(x: np.ndarray, kernel: np.ndarray) -> np.ndarray:
    out, _ = _run(x, kernel, trace=False)
    return out
